# revision 6
# baseline (speedup 1.0000x reference)
"""Trainium2 Bass kernel: sparse-FFN decode matvec (moe_routing).

Computes out[b, 0, j] = sum_d x[b, 0, d] * weight[indices[j], d]
for x [64, 1, 4096] f32, weight [11008, 4096] f32, indices [4403] int.

Strategy (tensor-parallel over the neuron axis, 8 NeuronCores):
  - host: weight -> bf16 once; stable-sort the indices and shard them by
    contiguous weight row-band (V/8 = 1376 rows per core), so each core
    only receives its own 11 MB weight band. Pad each shard to 640 = 5*128.
  - per core: 5x dma_gather(transpose=True) pulls 128 rows each directly
    into d-major (transposed) SBUF layout at 16-bit granularity; then 32
    bf16 matmuls per tile (K=128 d-chunks) accumulate out[64, 128] in PSUM.
  - host: concat the 8 shards (band order == sorted order), inverse-permute
    columns, reshape to [64, 1, 4403] f32.

Fallback paths (env BASS_MOE_PATH=indirect, or band overflow > 640):
  indirect_dma_start row gather + PE transposes; global sharding variant
  with the full weight on every core.
"""

import os

import numpy as np
import ml_dtypes

V = 11008          # loaded neurons (weight rows)
D = 4096           # hidden dim
B = 64             # batch
N_IDX = 4403       # selected core neurons
NCORES = 8
BAND = V // NCORES  # 1376 rows per core (banded sharding)
NT = 5             # 128-row gather tiles per core
NPC = NT * 128     # padded per-core neuron count (640)
DC = D // 128      # 32 d-chunks

_compiled = {}


def _build(nrows, path, reps=1):
    """Build + compile the SPMD program. nrows = weight rows per core
    (BAND for the banded fast path, V for the global fallback).
    reps > 1 repeats the whole body (for delta-timing benchmarks)."""
    import concourse.bacc as bacc
    import concourse.bass as bass
    import concourse.mybir as mybir
    import concourse.tile as tile
    from concourse.masks import make_identity

    f32 = mybir.dt.float32
    bf16 = mybir.dt.bfloat16
    i16 = mybir.dt.int16

    nc = bacc.Bacc(
        "TRN2",
        target_bir_lowering=False,
        debug=False,
        enable_asserts=False,
        num_devices=NCORES,
    )
    w = nc.dram_tensor("w", [nrows, D], bf16, kind="ExternalInput").ap()
    xt = nc.dram_tensor("xt", [128, DC * B], bf16, kind="ExternalInput").ap()
    idx = nc.dram_tensor("idx", [128, NT * 8], i16, kind="ExternalInput").ap()
    out = nc.dram_tensor("out", [B, NPC], f32, kind="ExternalOutput").ap()

    with tile.TileContext(nc) as tc:
        with (
            tc.tile_pool(name="const", bufs=1) as const_pool,
            tc.tile_pool(name="g", bufs=3) as g_pool,
            tc.tile_pool(name="wt", bufs=4) as wt_pool,
            tc.tile_pool(name="tps", bufs=3, space="PSUM") as ps_pool,
            tc.tile_pool(name="ops", bufs=2, space="PSUM") as ops_pool,
            tc.tile_pool(name="osb", bufs=1) as o_pool,
        ):
            xt_sb = const_pool.tile([128, DC * B], bf16)
            nc.sync.dma_start(xt_sb[:], xt[:])

            idx_sb = const_pool.tile([128, NT * 8], i16)
            nc.sync.dma_start(idx_sb[:], idx[:])

            out_sb = o_pool.tile([B, NPC], f32)

            if path == "gather":
                for t in range(NT * reps):
                    t = t % NT
                    g = g_pool.tile([128, DC * 128], bf16, tag="g")
                    nc.gpsimd.dma_gather(
                        g[:].rearrange("p (s n) -> p s n", n=128),
                        w[:],
                        idx_sb[:, t * 8 : (t + 1) * 8],
                        128,
                        128,
                        D,
                        transpose=True,
                    )
                    out_ps = ops_pool.tile([B, 128], f32, tag="ops")
                    for c in range(DC):
                        nc.tensor.matmul(
                            out_ps[:],
                            lhsT=xt_sb[:, c * B : (c + 1) * B],
                            rhs=g[:, c * 128 : (c + 1) * 128],
                            start=(c == 0),
                            stop=(c == DC - 1),
                        )
                    nc.vector.tensor_copy(
                        out_sb[:, t * 128 : (t + 1) * 128], out_ps[:]
                    )
            else:  # indirect + PE transpose fallback
                ident = const_pool.tile([128, 128], bf16)
                make_identity(nc, ident[:])
                idx32_sb = const_pool.tile([128, NT], mybir.dt.int32)
                # widen the wrapped int16 indices back to per-partition int32:
                # host also supplies a [128, NT] int32 layout in the last
                # columns of... simpler: reload from a second dram tensor.
                idx32 = nc.dram_tensor(
                    "idx32", [128, NT], mybir.dt.int32, kind="ExternalInput"
                ).ap()
                nc.sync.dma_start(idx32_sb[:], idx32[:])
                for t in range(NT):
                    wn = g_pool.tile([128, D], bf16, tag="wn")
                    nc.gpsimd.indirect_dma_start(
                        out=wn[:],
                        out_offset=None,
                        in_=w[:],
                        in_offset=bass.IndirectOffsetOnAxis(
                            ap=idx32_sb[:, t : t + 1], axis=0
                        ),
                    )
                    out_ps = ops_pool.tile([B, 128], f32, tag="ops")
                    for gi in range(8):
                        ps = ps_pool.tile([128, 512], bf16, tag="tps")
                        for j in range(4):
                            c = gi * 4 + j
                            nc.tensor.transpose(
                                ps[:, j * 128 : (j + 1) * 128],
                                wn[:, c * 128 : (c + 1) * 128],
                                ident[:],
                            )
                        wt = wt_pool.tile([128, 512], bf16, tag="wt")
                        if gi % 2 == 0:
                            nc.vector.tensor_copy(wt[:], ps[:])
                        else:
                            nc.scalar.copy(wt[:], ps[:])
                        for j in range(4):
                            c = gi * 4 + j
                            nc.tensor.matmul(
                                out_ps[:],
                                lhsT=xt_sb[:, c * B : (c + 1) * B],
                                rhs=wt[:, j * 128 : (j + 1) * 128],
                                start=(c == 0),
                                stop=(c == DC - 1),
                            )
                    nc.vector.tensor_copy(
                        out_sb[:, t * 128 : (t + 1) * 128], out_ps[:]
                    )

            nc.sync.dma_start(out[:], out_sb[:])

    nc.compile()
    return nc


def _get_compiled(nrows, path, reps=1):
    key = (nrows, path, reps)
    if key not in _compiled:
        _compiled[key] = _build(nrows, path, reps)
    return _compiled[key]


def _wrap_idx16(ids):
    """[NPC] int -> [128, NT*8] int16 wrapped layout for dma_gather:
    per 128-index gather block, unwrapped index j lives at [j % 16, j // 16]
    of a 16-partition block, replicated 8x down the partitions."""
    blocks = []
    for t in range(NT):
        blk = ids[t * 128 : (t + 1) * 128].astype(np.int16)
        wrapped = blk.reshape(8, 16).T  # [16, 8], col-major unwrap
        blocks.append(np.tile(wrapped, (8, 1)))  # [128, 8]
    return np.ascontiguousarray(np.concatenate(blocks, axis=1))


def _prep_xt(x):
    x = np.asarray(x, dtype=np.float32).reshape(B, D)
    xv = x.astype(ml_dtypes.bfloat16)  # round-to-nearest
    # [partition p, d-chunk c, batch b] = x[b, c*128+p]
    return np.ascontiguousarray(
        xv.T.reshape(DC, 128, B).transpose(1, 0, 2)
    ).reshape(128, DC * B)


def _prep_inputs(x, weight, indices):
    wbf = np.asarray(weight, dtype=np.float32).astype(ml_dtypes.bfloat16)
    indices = np.asarray(indices).astype(np.int64).reshape(N_IDX)
    xt_host = _prep_xt(x)

    perm = np.argsort(indices, kind="stable")
    sidx = indices[perm]

    path = os.environ.get("BASS_MOE_PATH", "gather")

    # banded sharding: core c serves sorted indices in [c*BAND, (c+1)*BAND)
    bounds = np.searchsorted(sidx, np.arange(NCORES + 1) * BAND)
    counts = np.diff(bounds)
    banded = counts.max() <= NPC

    in_maps = []
    if banded:
        nrows = BAND
        for c in range(NCORES):
            lo, hi = bounds[c], bounds[c + 1]
            ids = np.zeros(NPC, dtype=np.int64)
            ids[: hi - lo] = sidx[lo:hi] - c * BAND
            m = {
                "w": wbf[c * BAND : (c + 1) * BAND],
                "xt": xt_host,
                "idx": _wrap_idx16(ids),
            }
            if path != "gather":
                m["idx32"] = np.ascontiguousarray(
                    ids.reshape(NT, 128).T.astype(np.int32)
                )
            in_maps.append(m)
        counts = list(counts)
    else:
        # global fallback: equal shards, full weight everywhere
        nrows = V
        chunk = (N_IDX + NCORES - 1) // NCORES
        counts = []
        for c in range(NCORES):
            lo = c * chunk
            hi = min(lo + chunk, N_IDX)
            counts.append(hi - lo)
            ids = np.zeros(NPC, dtype=np.int64)
            ids[: hi - lo] = sidx[lo:hi]
            m = {"w": wbf, "xt": xt_host, "idx": _wrap_idx16(ids)}
            if path != "gather":
                m["idx32"] = np.ascontiguousarray(
                    ids.reshape(NT, 128).T.astype(np.int32)
                )
            in_maps.append(m)

    return in_maps, counts, perm, nrows, path


def _run(nrows, path, in_maps, trace=False):
    from concourse.bass_utils import run_bass_kernel_spmd

    nc = _get_compiled(nrows, path)
    kw = {"trace": True} if trace else {}
    return run_bass_kernel_spmd(nc, in_maps, core_ids=list(range(NCORES)), **kw)


def kernel(x, weight, indices, _trace=False):
    in_maps, counts, perm, nrows, path = _prep_inputs(x, weight, indices)
    res = _run(nrows, path, in_maps, trace=_trace)

    parts = [res.results[c]["out"][:, : counts[c]] for c in range(NCORES)]
    y_sorted = np.concatenate(parts, axis=1)  # [B, N_IDX] in sorted order
    y = np.empty((B, N_IDX), dtype=np.float32)
    y[:, perm] = y_sorted
    out = y.reshape(B, 1, N_IDX)
    if _trace:
        return out, res
    return out


# revision 9
# speedup vs baseline: 23.4785x; 23.4785x over previous
"""Trainium2 Bass kernel: sparse-FFN decode matvec (moe_routing).

Computes out[b, 0, j] = sum_d x[b, 0, d] * weight[indices[j], d]
for x [64, 1, 4096] f32, weight [11008, 4096] f32, indices [4403] int.

Strategy (tensor-parallel over the neuron axis, 8 NeuronCores):
  - host: weight -> bf16 once; stable-sort the indices and shard them by
    contiguous weight row-band (V/8 = 1376 rows per core), so each core
    only receives its own 11 MB weight band. Pad each shard to 640 = 5*128.
  - per core: 5x dma_gather(transpose=True) pulls 128 rows each directly
    into d-major (transposed) SBUF layout at 16-bit granularity; then 32
    bf16 matmuls per tile (K=128 d-chunks) accumulate out[64, 128] in PSUM.
  - host: concat the 8 shards (band order == sorted order), inverse-permute
    columns, reshape to [64, 1, 4403] f32.

Fallback paths (env BASS_MOE_PATH=indirect, or band overflow > 640):
  indirect_dma_start row gather + PE transposes; global sharding variant
  with the full weight on every core.
"""

import os

import numpy as np
import ml_dtypes

V = 11008          # loaded neurons (weight rows)
D = 4096           # hidden dim
B = 64             # batch
N_IDX = 4403       # selected core neurons
NCORES = 8
BAND = V // NCORES  # 1376 rows per core (banded sharding)
NT = 5             # 128-row gather tiles per core
NPC = NT * 128     # padded per-core neuron count (640)
DC = D // 128      # 32 d-chunks

_compiled = {}


def _build(nrows, path, reps=1):
    """Build + compile the SPMD program. nrows = weight rows per core
    (BAND for the banded fast path, V for the global fallback).
    reps > 1 repeats the whole body (for delta-timing benchmarks)."""
    import concourse.bacc as bacc
    import concourse.bass as bass
    import concourse.mybir as mybir
    import concourse.tile as tile
    from concourse.masks import make_identity

    f32 = mybir.dt.float32
    bf16 = mybir.dt.bfloat16
    i16 = mybir.dt.int16

    nc = bacc.Bacc(
        "TRN2",
        target_bir_lowering=False,
        debug=False,
        enable_asserts=False,
        num_devices=NCORES,
    )
    w = nc.dram_tensor("w", [nrows, D], bf16, kind="ExternalInput").ap()
    xt = nc.dram_tensor("xt", [128, DC * B], bf16, kind="ExternalInput").ap()
    idx = nc.dram_tensor("idx", [128, NT * 8], i16, kind="ExternalInput").ap()
    out = nc.dram_tensor("out", [B, NPC], f32, kind="ExternalOutput").ap()

    with tile.TileContext(nc) as tc:
        with (
            tc.tile_pool(name="const", bufs=1) as const_pool,
            tc.tile_pool(name="g", bufs=3) as g_pool,
            tc.tile_pool(name="wt", bufs=4) as wt_pool,
            tc.tile_pool(name="tps", bufs=3, space="PSUM") as ps_pool,
            tc.tile_pool(name="ops", bufs=2, space="PSUM") as ops_pool,
            tc.tile_pool(name="osb", bufs=2) as o_pool,
        ):
            xt_sb = const_pool.tile([128, DC * B], bf16)
            nc.sync.dma_start(xt_sb[:], xt[:])

            idx_sb = const_pool.tile([128, NT * 8], i16)
            nc.sync.dma_start(idx_sb[:], idx[:])

            if path == "gather":
                for t in range(NT * reps):
                    t = t % NT
                    g = g_pool.tile([128, DC * 128], bf16, tag="g")
                    nc.gpsimd.dma_gather(
                        g[:].rearrange("p (s n) -> p s n", n=128),
                        w[:],
                        idx_sb[:, t * 8 : (t + 1) * 8],
                        128,
                        128,
                        D,
                        transpose=True,
                    )
                    out_ps = ops_pool.tile([B, 128], f32, tag="ops")
                    for c in range(DC):
                        nc.tensor.matmul(
                            out_ps[:],
                            lhsT=xt_sb[:, c * B : (c + 1) * B],
                            rhs=g[:, c * 128 : (c + 1) * 128],
                            start=(c == 0),
                            stop=(c == DC - 1),
                        )
                    ot = o_pool.tile([B, 128], f32, tag="ot")
                    nc.vector.tensor_copy(ot[:], out_ps[:])
                    nc.sync.dma_start(out[:, t * 128 : (t + 1) * 128], ot[:])
            else:  # indirect + PE transpose fallback
                out_sb = const_pool.tile([B, NPC], f32)
                ident = const_pool.tile([128, 128], bf16)
                make_identity(nc, ident[:])
                idx32_sb = const_pool.tile([128, NT], mybir.dt.int32)
                # widen the wrapped int16 indices back to per-partition int32:
                # host also supplies a [128, NT] int32 layout in the last
                # columns of... simpler: reload from a second dram tensor.
                idx32 = nc.dram_tensor(
                    "idx32", [128, NT], mybir.dt.int32, kind="ExternalInput"
                ).ap()
                nc.sync.dma_start(idx32_sb[:], idx32[:])
                for t in range(NT):
                    wn = g_pool.tile([128, D], bf16, tag="wn")
                    nc.gpsimd.indirect_dma_start(
                        out=wn[:],
                        out_offset=None,
                        in_=w[:],
                        in_offset=bass.IndirectOffsetOnAxis(
                            ap=idx32_sb[:, t : t + 1], axis=0
                        ),
                    )
                    out_ps = ops_pool.tile([B, 128], f32, tag="ops")
                    for gi in range(8):
                        ps = ps_pool.tile([128, 512], bf16, tag="tps")
                        for j in range(4):
                            c = gi * 4 + j
                            nc.tensor.transpose(
                                ps[:, j * 128 : (j + 1) * 128],
                                wn[:, c * 128 : (c + 1) * 128],
                                ident[:],
                            )
                        wt = wt_pool.tile([128, 512], bf16, tag="wt")
                        if gi % 2 == 0:
                            nc.vector.tensor_copy(wt[:], ps[:])
                        else:
                            nc.scalar.copy(wt[:], ps[:])
                        for j in range(4):
                            c = gi * 4 + j
                            nc.tensor.matmul(
                                out_ps[:],
                                lhsT=xt_sb[:, c * B : (c + 1) * B],
                                rhs=wt[:, j * 128 : (j + 1) * 128],
                                start=(c == 0),
                                stop=(c == DC - 1),
                            )
                    nc.vector.tensor_copy(
                        out_sb[:, t * 128 : (t + 1) * 128], out_ps[:]
                    )

                nc.sync.dma_start(out[:], out_sb[:])

    nc.compile()
    return nc


def _get_compiled(nrows, path, reps=1):
    key = (nrows, path, reps)
    if key not in _compiled:
        _compiled[key] = _build(nrows, path, reps)
    return _compiled[key]


def _wrap_idx16(ids):
    """[NPC] int -> [128, NT*8] int16 wrapped layout for dma_gather:
    per 128-index gather block, unwrapped index j lives at [j % 16, j // 16]
    of a 16-partition block, replicated 8x down the partitions."""
    blocks = []
    for t in range(NT):
        blk = ids[t * 128 : (t + 1) * 128].astype(np.int16)
        wrapped = blk.reshape(8, 16).T  # [16, 8], col-major unwrap
        blocks.append(np.tile(wrapped, (8, 1)))  # [128, 8]
    return np.ascontiguousarray(np.concatenate(blocks, axis=1))


def _prep_xt(x):
    x = np.asarray(x, dtype=np.float32).reshape(B, D)
    xv = x.astype(ml_dtypes.bfloat16)  # round-to-nearest
    # [partition p, d-chunk c, batch b] = x[b, c*128+p]
    return np.ascontiguousarray(
        xv.T.reshape(DC, 128, B).transpose(1, 0, 2)
    ).reshape(128, DC * B)


def _prep_inputs(x, weight, indices):
    wbf = np.asarray(weight, dtype=np.float32).astype(ml_dtypes.bfloat16)
    indices = np.asarray(indices).astype(np.int64).reshape(N_IDX)
    xt_host = _prep_xt(x)

    perm = np.argsort(indices, kind="stable")
    sidx = indices[perm]

    path = os.environ.get("BASS_MOE_PATH", "gather")

    # banded sharding: core c serves sorted indices in [c*BAND, (c+1)*BAND)
    bounds = np.searchsorted(sidx, np.arange(NCORES + 1) * BAND)
    counts = np.diff(bounds)
    banded = counts.max() <= NPC

    in_maps = []
    if banded:
        nrows = BAND
        for c in range(NCORES):
            lo, hi = bounds[c], bounds[c + 1]
            ids = np.zeros(NPC, dtype=np.int64)
            ids[: hi - lo] = sidx[lo:hi] - c * BAND
            m = {
                "w": wbf[c * BAND : (c + 1) * BAND],
                "xt": xt_host,
                "idx": _wrap_idx16(ids),
            }
            if path != "gather":
                m["idx32"] = np.ascontiguousarray(
                    ids.reshape(NT, 128).T.astype(np.int32)
                )
            in_maps.append(m)
        counts = list(counts)
    else:
        # global fallback: equal shards, full weight everywhere
        nrows = V
        chunk = (N_IDX + NCORES - 1) // NCORES
        counts = []
        for c in range(NCORES):
            lo = c * chunk
            hi = min(lo + chunk, N_IDX)
            counts.append(hi - lo)
            ids = np.zeros(NPC, dtype=np.int64)
            ids[: hi - lo] = sidx[lo:hi]
            m = {"w": wbf, "xt": xt_host, "idx": _wrap_idx16(ids)}
            if path != "gather":
                m["idx32"] = np.ascontiguousarray(
                    ids.reshape(NT, 128).T.astype(np.int32)
                )
            in_maps.append(m)

    return in_maps, counts, perm, nrows, path


def _run(nrows, path, in_maps, trace=False):
    from concourse.bass_utils import run_bass_kernel_spmd

    nc = _get_compiled(nrows, path)
    kw = {"trace": True} if trace else {}
    return run_bass_kernel_spmd(nc, in_maps, core_ids=list(range(NCORES)), **kw)


def kernel(x, weight, indices, _trace=False):
    in_maps, counts, perm, nrows, path = _prep_inputs(x, weight, indices)
    res = _run(nrows, path, in_maps, trace=_trace)

    parts = [res.results[c]["out"][:, : counts[c]] for c in range(NCORES)]
    y_sorted = np.concatenate(parts, axis=1)  # [B, N_IDX] in sorted order
    y = np.empty((B, N_IDX), dtype=np.float32)
    y[:, perm] = y_sorted
    out = y.reshape(B, 1, N_IDX)
    if _trace:
        return out, res
    return out
